# revision 21
# baseline (speedup 1.0000x reference)
"""Trainium2 Bass kernel for BondLengthConstraintEnergy.

Contract: kernel(**inputs) takes FULL unsharded inputs (as produced by the
problem's setup_inputs) and returns the FULL output [B, NCH, NRES, n_alt].

Strategy
--------
The input layout produced by setup_inputs is canonical: atom i corresponds to
(b, ch, r, a) = unravel(i) over (32, 8, 8192, 3), so the (b,ch,r,atom)->row
lookup table is exactly arange, every peptide bond (b,ch,r)->(b,ch,r+1) is
present, and the per-residue-type mean/std tables have identical rows.  Under
those conditions (verified on the host each call) the whole computation
collapses to a pure streaming stencil over coords:

  per bond r (residue r, r+1 in the same chain):
    b = C_r - CA_r          (v_cac_c)
    w = N_{r+1} - C_r       (v_cn)
    a = CA_{r+1} - N_{r+1}  (v_nca_n)
    ang1 = angle(w, a), ang2 = angle(b, -w), len = |w|
    lp_i  = min(d_i^2 / (2 var_i), -ln(EPS) - ln(sqrt(2 pi var_i)))
    out[b,ch,r,0] = (lp0+lp1+lp2) * (1 - tanh(-weight))

Angles are computed without any acos on device via
    theta = pi/2 - atan(dot / sqrt(|u|^2 |v|^2 - dot^2))
which is exact for theta in (0, pi) and numerically great in the region where
the gaussian is not clipped.

Engine budget (cost model): Act gets only Square/Relu/Sqrt/Arctan work (all
activation functions grouped so only 2 table loads fire), the length term is
computed square-free on DVE via lp0 = min(K0*(na2 - 2 m0 na), C0') + const,
and the remaining elementwise work is balanced across Pool and DVE.

Sharding: data-parallel over batch, 4 structures per core, no communication.
Each core streams 9.4 MB of coords and writes 1 MB of energies.

If the host-side structure checks fail (inputs are not canonical), we fall
back to a faithful numpy implementation of the reference.
"""

import os
import sys

import numpy as np

for _p in ("/opt/trn_rl_repo",):
    if os.path.isdir(_p) and _p not in sys.path:
        sys.path.insert(0, _p)

# ---------------------------------------------------------------- constants
B, NCH, NRES, APR = 32, 8, 8192, 3
N_ATOMS = B * NCH * NRES * APR
NCORES = 8
B_PER_CORE = B // NCORES
RES_PER_CORE = B_PER_CORE * NCH * NRES          # 262144
ATOMS_PER_CORE = RES_PER_CORE * APR
P = 128                                          # SBUF partitions
RES_PER_PART = RES_PER_CORE // P                 # 2048
EPS = 1e-8
NEG_LOG_EPS = 18.420680743952367                 # -ln(1e-8)
R2D = 180.0 / np.pi
TINY = 1e-38

# benign pad residue (N=(0,0,0), CA=(1,0,0), C=(2,0,0)) keeps the one
# out-of-range halo bond finite; its output is overwritten on the host.
_PAD_RESIDUE = np.array([0, 0, 0, 1, 0, 0, 2, 0, 0], dtype=np.float32)

_PROGRAM = None


# ---------------------------------------------------------------- device IR
def _build_program(reps=1, cfg=None):
    """Build + compile the per-core Bass/Tile program (identical on all cores).

    reps>1 wraps the whole body in a device-side loop — used only by the
    timing harness to amplify kernel time over dispatch/transfer noise.
    """
    import concourse.bacc as bacc
    import concourse.bass as bass
    import concourse.mybir as mybir
    import concourse.tile as tile

    import bass_rust

    cfg = dict(cfg or {})
    W = cfg.get("W", 512)
    tiles = cfg.get("tiles")
    if tiles is None:
        tiles = [W] * (RES_PER_PART // W)
    assert sum(tiles) == RES_PER_PART
    offs = [0]
    for w_ in tiles:
        offs.append(offs[-1] + w_)
    NT = len(tiles)
    xbufs = cfg.get("xbufs", 2)
    dbufs = cfg.get("dbufs", 2)
    sbufs = cfg.get("sbufs", 2)
    pbufs = cfg.get("pbufs", 1)
    midbufs = cfg.get("midbufs", 1)
    ph2bufs = cfg.get("ph2bufs", 2)
    fence = cfg.get("fence", True)
    use_div = cfg.get("div", False)
    f16 = cfg.get("f16", True)
    f16_ratio = cfg.get("f16_ratio", True) and f16
    f16_tail = cfg.get("f16_tail", True) and f16
    out16 = cfg.get("out16", False) and f16
    kd = cfg.get("kd", 0.72)        # fraction of d-sub on DVE (rest pool)
    dsq_eng = cfg.get("dsq_eng", "act")
    dots_eng = cfg.get("dots_eng", "pool")  # pool|dve
    r2_eng = cfg.get("r2_eng", "pool")
    q_eng = cfg.get("q_eng", "pool")
    sum_eng = cfg.get("sum_eng", "dve")

    dt = mybir.dt
    Alu = mybir.AluOpType
    Act = mybir.ActivationFunctionType
    f16r = dt.float16 if f16_ratio else dt.float32
    f16l = dt.float16 if f16_tail else dt.float32
    outdt = dt.float16 if out16 else dt.float32

    nc = bacc.Bacc(
        "TRN2",
        target_bir_lowering=False,
        debug=False,
        enable_asserts=False,
        num_devices=NCORES,
    )

    xin = nc.dram_tensor("xin", [(RES_PER_CORE + 1) * 9], dt.float32,
                         kind="ExternalInput")
    cst = nc.dram_tensor("consts", [P, 16], dt.float32, kind="ExternalInput")
    out = nc.dram_tensor("out", [RES_PER_CORE], outdt,
                         kind="ExternalOutput")

    def pool_stt(out_, a, b, op):
        # NB: TensorScalarPtr fails the hardware ISA check on Pool, so
        # gpsimd work must go through plain TensorTensor.
        nc.gpsimd.tensor_tensor(out_, a, b, op=op)

    def dve_tt(out_, a, b, op):
        nc.vector.tensor_tensor(out_, a, b, op=op)

    with tile.TileContext(nc) as tc:
        with (
            tc.tile_pool(name="cpool", bufs=1) as cpool,
            tc.tile_pool(name="xpool", bufs=xbufs) as xpool,
            tc.tile_pool(name="dpool", bufs=dbufs) as dpool,
            tc.tile_pool(name="spool", bufs=sbufs) as spool,
            tc.tile_pool(name="ppool", bufs=pbufs) as ppool,
            tc.tile_pool(name="mid", bufs=midbufs) as mid,
            tc.tile_pool(name="xph", bufs=1) as xph,      # crosses phase bound
            tc.tile_pool(name="ph2", bufs=ph2bufs) as ph2,
        ):
            ctile = cpool.tile([P, 16], dt.float32, tag="c")
            nc.sync.dma_start(ctile[:, :], cst.ap())
            warm = cpool.tile([P, 1], dt.float32, tag="warm")
            nc.scalar.activation(warm[:, :], ctile[:, 7:8], Act.Sqrt)
            c_bias1 = ctile[:, 0:1]
            c_bias2 = ctile[:, 1:2]
            c_k1 = ctile[:, 2:3]
            c_c1 = ctile[:, 3:4]
            c_nm0 = ctile[:, 6:7]
            c_k0 = ctile[:, 7:8]
            c_c0 = ctile[:, 8:9]

            def _body():
                groups = cfg.get("ph2_batches", cfg.get("fence_groups", 2))
                tgroups = cfg.get("fence_groups", 2)
                bnd = [round(g * NT / groups) for g in range(groups + 1)]
                tbnd = [round(g * NT / tgroups) for g in range(tgroups + 1)]
                tile2tg = [max(i for i in range(tgroups) if tbnd[i] <= t)
                           for t in range(NT)]
                batch2tg = [tile2tg[bnd[g]] for g in range(groups)]
                gof = [offs[b] for b in bnd]          # residue offset per group
                sqrt_insts = [[] for _ in range(tgroups)]
                atan_insts = [[] for _ in range(tgroups)]
                gratio = []
                glp0 = []
                for g in range(groups):
                    GW = gof[g + 1] - gof[g]
                    gratio.append(xph.tile([P, 2 * GW], f16r,
                                            tag=f"ratio{g}",
                                            name=f"ratio{g}"))
                    glp0.append(xph.tile([P, GW], f16l, tag=f"lp0x{g}",
                                         name=f"lp0x{g}"))

                def emit_ph2(g):
                    GW = gof[g + 1] - gof[g]
                    ratio, lp0x = gratio[g], glp0[g]
                    h = ph2.tile([P, 2 * GW], f16r, tag="h")
                    h_inst = nc.scalar.activation(h[:, :], ratio[:, :],
                                                  Act.Arctan)
                    atan_insts[batch2tg[g]].append(h_inst)
                    hv = h[:, :].rearrange("p (w t) -> p w t", t=2)
                    # slot t=1 -> dot1/ang1 (bias1), slot t=0 -> dot2/ang2
                    sq12 = ph2.tile([P, 2 * GW], f16l, tag="sq12")
                    i_a = nc.scalar.activation(sq12[:, 0:GW], hv[:, :, 1],
                                               Act.Square, bias=c_bias1)
                    i_b = nc.scalar.activation(sq12[:, GW:2 * GW], hv[:, :, 0],
                                               Act.Square, bias=c_bias2)
                    atan_insts[batch2tg[g]].extend([i_a, i_b])
                    lp12 = ph2.tile([P, 2 * GW], f16l, tag="lp12")
                    nc.vector.tensor_scalar(lp12[:, :], sq12[:, :], c_k1,
                                            c_c1, op0=Alu.mult, op1=Alu.min)
                    s01 = ph2.tile([P, GW], f16l, tag="s01")
                    val = ph2.tile([P, GW], outdt, tag="val")
                    if sum_eng == "dve":
                        dve_tt(s01[:, :], lp0x[:, :], lp12[:, 0:GW], Alu.add)
                        dve_tt(val[:, :], s01[:, :], lp12[:, GW:2 * GW],
                               Alu.add)
                    else:
                        pool_stt(s01[:, :], lp0x[:, :], lp12[:, 0:GW],
                                 Alu.add)
                        pool_stt(val[:, :], s01[:, :], lp12[:, GW:2 * GW],
                                 Alu.add)
                    dst = bass.AP(out, P * gof[g], [[GW, P], [1, GW]])
                    nc.sync.dma_start(dst, val[:, :])

                def emit_ph1(t, g):
                    W = tiles[t]
                    FW = 9 * W
                    XW = 9 * (W + 1)
                    base = P * offs[t]
                    co = offs[t] - gof[g]          # column offset in group
                    x = xpool.tile([P, XW], dt.float32, tag="x")
                    if t == 0 and cfg.get("dma_split_first", True):
                        h1 = (XW // 2) // 4 * 4
                        nc.sync.dma_start(
                            x[:, 0:h1],
                            bass.AP(xin, base * 9, [[FW, P], [1, h1]]))
                        nc.sync.dma_start(
                            x[:, h1:XW],
                            bass.AP(xin, base * 9 + h1,
                                    [[FW, P], [1, XW - h1]]))
                    else:
                        src = bass.AP(xin, base * 9, [[FW, P], [1, XW]])
                        nc.sync.dma_start(x[:, :], src)

                    # D[i] = X[i+6] - X[i+3]; per group j (bond j):
                    #   D[9j+0..2]=v_cac, D[9j+3..5]=v_cn, D[9j+6..8]=v_nca
                    d = dpool.tile([P, FW], dt.float32, tag="d")
                    k = int(FW * kd)
                    if t == 0 and cfg.get("dma_split_first", True):
                        h1 = (XW // 2) // 4 * 4
                        ka = min(h1 - 6, k)
                        if ka > 0:
                            nc.vector.tensor_sub(d[:, 0:ka], x[:, 6:6 + ka],
                                                 x[:, 3:3 + ka])
                        if ka < k:
                            nc.vector.tensor_sub(d[:, ka:k],
                                                 x[:, 6 + ka:6 + k],
                                                 x[:, 3 + ka:3 + k])
                    elif k > 0:
                        nc.vector.tensor_sub(d[:, 0:k], x[:, 6:6 + k],
                                             x[:, 3:3 + k])
                    if k < FW:
                        nc.gpsimd.tensor_sub(d[:, k:FW], x[:, 6 + k:6 + FW],
                                             x[:, 3 + k:3 + FW])

                    # squares of all components (Act; Square is in every
                    # activation table so it never forces a table switch)
                    s = spool.tile([P, FW], dt.float32, tag="s")
                    nc.scalar.activation(s[:, :], d[:, :], Act.Square)

                    # P6[6j+m] = D[9j+m]*D[9j+m+3], m=0..5
                    #   m=0..2 -> v_cac.v_cn (dot2), m=3..5 -> v_cn.v_nca (dot1)
                    d3 = d[:, :].rearrange("p (w k) -> p w k", k=9)
                    p6 = ppool.tile([P, 6 * W], dt.float32, tag="p6")
                    p6v = p6[:, :].rearrange("p (w k) -> p w k", k=6)
                    dve_tt(p6v, d3[:, :, 0:6], d3[:, :, 3:9], Alu.mult)

                    # windowed 3-sums of squares: R2[j] = (nc2, na2, nb2)
                    sv = s[:, :].rearrange("p (w t k) -> p w t k", t=3, k=3)
                    r2 = mid.tile([P, 3 * W], dt.float32, tag="r2")
                    r2v = r2[:, :].rearrange("p (w t) -> p w t", t=3)
                    r2f = pool_stt if r2_eng == "pool" else dve_tt
                    r2f(r2v, sv[:, :, :, 0], sv[:, :, :, 1], Alu.add)
                    r2f(r2v, r2v, sv[:, :, :, 2], Alu.add)

                    # dots: DOTS[j] = (dot2, dot1)
                    pv = p6[:, :].rearrange("p (w t k) -> p w t k", t=2, k=3)
                    dots = mid.tile([P, 2 * W], dt.float32, tag="dots")
                    dotsv = dots[:, :].rearrange("p (w t) -> p w t", t=2)
                    dotf = pool_stt if dots_eng == "pool" else dve_tt
                    dotf(dotsv, pv[:, :, :, 0], pv[:, :, :, 1], Alu.add)
                    dotf(dotsv, dotsv, pv[:, :, :, 2], Alu.add)

                    # q interleaved to match DOTS: (q2, q1) = (nc2*na2, na2*nb2)
                    r2t = r2[:, :].rearrange("p (w t) -> p w t", t=3)
                    q = mid.tile([P, 2 * W], dt.float32, tag="q")
                    qv = q[:, :].rearrange("p (w t) -> p w t", t=2)
                    qf = pool_stt if q_eng == "pool" else dve_tt
                    qf(qv, r2t[:, :, 0:2], r2t[:, :, 1:3], Alu.mult)

                    # v = q - dot^2, clamped positive
                    dsq = mid.tile([P, 2 * W], dt.float32, tag="dsq")
                    if dsq_eng == "act":
                        nc.scalar.activation(dsq[:, :], dots[:, :],
                                             Act.Square)
                    else:
                        dve_tt(dsq[:, :], dots[:, :], dots[:, :], Alu.mult)
                    v = mid.tile([P, 2 * W], dt.float32, tag="v")
                    kv = int(2 * W * cfg.get("kv", 1.0)) // 2 * 2
                    if kv > 0:
                        nc.vector.tensor_sub(v[:, 0:kv], q[:, 0:kv],
                                             dsq[:, 0:kv])
                    if kv < 2 * W:
                        nc.gpsimd.tensor_sub(v[:, kv:2 * W], q[:, kv:2 * W],
                                             dsq[:, kv:2 * W])
                    vc = mid.tile([P, 2 * W], dt.float32, tag="vc", bufs=cfg.get("b_vc", 1))
                    if cfg.get("vc_eng", "dve") == "act":
                        nc.scalar.activation(vc[:, :], v[:, :], Act.Relu)
                    else:
                        nc.vector.tensor_scalar(vc[:, :], v[:, :], TINY, None,
                                                op0=Alu.max)

                    # sqrt(v); na = sqrt(na2); d0sq = (na - m0)^2
                    sq = mid.tile([P, 2 * W], dt.float32, tag="sq", bufs=cfg.get("b_sq", 1))
                    i_sq = nc.scalar.activation(sq[:, :], vc[:, :], Act.Sqrt)
                    na = mid.tile([P, W], dt.float32, tag="na", bufs=cfg.get("b_na", 1))
                    i_na = nc.scalar.activation(na[:, :], r2t[:, :, 1],
                                                Act.Sqrt)
                    sqrt_insts[tile2tg[t]].extend([i_sq, i_na])
                    d0sq = mid.tile([P, W], dt.float32, tag="d0sq", bufs=cfg.get("b_d0sq", 1))
                    nc.scalar.activation(d0sq[:, :], na[:, :], Act.Square,
                                         bias=c_nm0)
                    nc.vector.tensor_scalar(glp0[g][:, co:co + W], d0sq[:, :],
                                            c_k0, c_c0, op0=Alu.mult,
                                            op1=Alu.min)

                    # ratio = dot / sqrt(v)   (= cot(theta)), clamped to the
                    # scalar engine's arctan domain [-pi/2, pi/2].  Saturation
                    # maps theta into [32.5, 147.5] deg, far inside the
                    # gaussian clip zone, so clamped bonds stay exact.
                    rr = mid.tile([P, 2 * W], f16r, tag="rr", bufs=cfg.get("b_rr", 1))
                    if use_div:
                        dve_tt(rr[:, :], dots[:, :], sq[:, :], Alu.divide)
                    else:
                        rv = mid.tile([P, 2 * W], dt.float32, tag="rv")
                        nc.vector.reciprocal(rv[:, :], sq[:, :])
                        dve_tt(rr[:, :], dots[:, :], rv[:, :], Alu.mult)
                    nc.vector.tensor_scalar(gratio[g][:, 2 * co:2 * co + 2 * W],
                                            rr[:, :], 1.5703125, -1.5703125,
                                            op0=Alu.min, op1=Alu.max)

                for g in range(groups):
                    for t in range(bnd[g], bnd[g + 1]):
                        emit_ph1(t, g)
                    emit_ph2(g)

                # table fence: only Sqrt and Arctan are table-critical
                # (Square/Relu exist in every activation table).  Chain
                # sqrt-group <- atan-group alternately so the compiled Act
                # order needs only ngroups*2 table loads while keeping the
                # serial tail short.
                if fence:
                    for g in range(tgroups):
                        for a in atan_insts[g]:
                            for sI in sqrt_insts[g]:
                                bass_rust.add_dep_helper(
                                    a.ins, sI.ins, reason="fence sqrt<atan")
                        if g + 1 < tgroups:
                            for sI in sqrt_insts[g + 1]:
                                for a in atan_insts[g]:
                                    bass_rust.add_dep_helper(
                                        sI.ins, a.ins,
                                        reason="fence atan<next sqrt")

            if reps == 1:
                _body()
            else:
                with tc.For_i(0, reps, 1):
                    _body()

    nc.compile()
    return nc


_FAST_CFG = {"ph2_batches": 4, "fence_groups": 2, "midbufs": 2, "sbufs": 1, "kd": 0.8, "dots_eng": "dve"}


def _get_program():
    global _PROGRAM
    if _PROGRAM is None:
        _PROGRAM = _build_program(cfg=_FAST_CFG)
    return _PROGRAM


# ---------------------------------------------------------------- host side
def _make_consts(mean_row, std_row, weight0):
    m = np.asarray(mean_row, dtype=np.float64)
    s = np.asarray(std_row, dtype=np.float64)
    f = 1.0 - np.tanh(-float(weight0))
    var = s * s
    clip = NEG_LOG_EPS - 0.5 * np.log(2.0 * np.pi * var)
    k0 = 1.0 / (2.0 * var[0]) * f
    c0 = clip[0] * f
    c = np.zeros(16, dtype=np.float64)
    c[0] = (m[1] - 90.0) / R2D                    # bias1
    c[1] = (90.0 - m[2]) / R2D                    # bias2
    c[2] = (R2D * R2D) / (2.0 * var[1]) * f       # K1'
    c[3] = clip[1] * f                            # C1'
    c[4] = (R2D * R2D) / (2.0 * var[2]) * f       # K2'
    c[5] = clip[2] * f                            # C2'
    c[6] = -m[0]                                  # -mean_len
    c[7] = k0                                     # K0'
    c[8] = c0                                     # C0'
    c[9] = -2.0 * m[0]                            # -2*m0
    c[10] = c0 - k0 * m[0] * m[0]                 # C0' - K0'*m0^2
    c[11] = k0 * m[0] * m[0]                      # K0'*m0^2
    return np.tile(c.astype(np.float32), (P, 1))


def _is_canonical(ad, coords, mean, std):
    if ad.shape != (N_ATOMS, 5) or coords.shape != (N_ATOMS, 3):
        return False
    if mean.shape != (20, 3) or std.shape != (20, 3):
        return False
    if not (np.all(mean == mean[0:1]) and np.all(std == std[0:1])):
        return False
    if not np.all(std[0] > 0):
        return False
    if std[0, 1] != std[0, 2]:
        return False
    a5 = ad.reshape(B, NCH, NRES, APR, 5)
    if not np.all(a5[..., 0] == np.arange(B, dtype=ad.dtype)[:, None, None, None]):
        return False
    if not np.all(a5[..., 1] == np.arange(NCH, dtype=ad.dtype)[:, None, None]):
        return False
    if not np.all(a5[..., 2] == np.arange(NRES, dtype=ad.dtype)[:, None]):
        return False
    if not np.all(a5[..., 4] == np.arange(APR, dtype=ad.dtype)):
        return False
    if not np.isfinite(coords).all() or np.abs(coords).max() >= 1e4:
        return False
    # all bond-geometry norms must clear the reference's EPS mask, so the
    # device kernel can skip mask arithmetic entirely
    r = coords.reshape(B, NCH, NRES, 9)
    w = r[:, :, 1:, 0:3] - r[:, :, :-1, 6:9]
    a = r[:, :, 1:, 3:6] - r[:, :, 1:, 0:3]
    bb = r[:, :, :-1, 6:9] - r[:, :, :-1, 3:6]
    mn = min(
        (w * w).sum(-1).min(),
        (a * a).sum(-1).min(),
        (bb * bb).sum(-1).min(),
    )
    return bool(mn > 1.1e-16)


def _run_fast(coords, mean, std, weight, n_alt):
    from concourse import bass_utils

    assert std[0][1] == std[0][2], "merged lp12 requires equal angle stds"
    nc = _get_program()
    consts = _make_consts(mean[0], std[0], weight[0])
    cflat = np.ascontiguousarray(coords.reshape(-1), dtype=np.float32)
    in_maps = []
    for c in range(NCORES):
        shard = np.empty(((RES_PER_CORE + 1) * 9,), dtype=np.float32)
        shard[:-9] = cflat[c * ATOMS_PER_CORE * 3:(c + 1) * ATOMS_PER_CORE * 3]
        shard[-9:] = _PAD_RESIDUE
        in_maps.append({"xin": shard, "consts": consts})

    res = bass_utils.run_bass_kernel_spmd(nc, in_maps,
                                          core_ids=list(range(NCORES)))
    parts = [np.asarray(res.results[c]["out"]).astype(np.float32)
             for c in range(NCORES)]
    e = np.concatenate(parts).reshape(B, NCH, NRES)
    e[:, :, NRES - 1] = 0.0          # no bond out of the last residue
    full = np.zeros((B, NCH, NRES, n_alt), dtype=np.float32)
    full[..., 0] = e
    return full


# ------------------------------------------------------------ numpy fallback
def _fallback(ad, coords, alternatives, weight, mean, std):
    """Faithful numpy port of the jax reference (incl. OOB drop/clamp)."""
    n_alt = alternatives.shape[-1]
    batch, chain, resnum = ad[:, 0], ad[:, 1], ad[:, 2]
    resname, at_name = ad[:, 3], ad[:, 4]
    n = ad.shape[0]

    table = np.full((B, NCH, NRES, APR), -1, dtype=np.int32)
    ok = ((batch >= 0) & (batch < B) & (chain >= 0) & (chain < NCH)
          & (resnum >= 0) & (resnum < NRES) & (at_name >= 0) & (at_name < APR))
    idx = np.arange(n, dtype=np.int32)
    table[batch[ok], chain[ok], resnum[ok], at_name[ok]] = idx[ok]

    c_idx = table[:, :, :-1, 2].reshape(-1)
    n_idx = table[:, :, 1:, 0].reshape(-1)
    cac_idx = table[:, :, :-1, 1].reshape(-1)
    can_idx = table[:, :, 1:, 1].reshape(-1)
    valid_idx = (c_idx >= 0) & (n_idx >= 0) & (cac_idx >= 0) & (can_idx >= 0)
    safe = lambda i: np.where(i >= 0, i, 0)

    co = coords.astype(np.float32)
    c_xyz = co[safe(c_idx)]
    n_xyz = co[safe(n_idx)]
    cac_xyz = co[safe(cac_idx)]
    can_xyz = co[safe(can_idx)]

    v_cn = n_xyz - c_xyz
    v_nca = can_xyz - n_xyz
    v_cac = c_xyz - cac_xyz

    def ang_deg(a, b):
        na = np.sqrt((a * a).sum(-1))
        nb = np.sqrt((b * b).sum(-1))
        mask = (na > EPS) & (nb > EPS)
        cos = np.clip((a * b).sum(-1) / (na * nb + EPS), -1.0, 1.0)
        return np.degrees(np.arccos(cos)).astype(np.float32), mask

    ang1, m1 = ang_deg(v_cn, v_nca)
    ang2, m2 = ang_deg(v_cac, -v_cn)
    bond_len = np.sqrt((v_cn * v_cn).sum(-1))
    valid = valid_idx & m1 & m2

    geom = np.stack([bond_len, ang1, ang2], axis=-1)
    seq = np.clip(resname[safe(c_idx)], 0, 19)
    var = (std.astype(np.float32)[seq]) ** 2
    denom = np.sqrt(2.0 * np.pi * var).astype(np.float32)
    num = np.exp(-((geom - mean.astype(np.float32)[seq]) ** 2) / (2.0 * var))
    log_prob = -(np.log(np.clip(num / denom, EPS, None)) + np.log(denom))
    scores = log_prob.sum(-1)

    f = np.float32(1.0 - np.tanh(-np.float32(weight[0])))
    val = np.where(valid, scores * f, 0.0).astype(np.float32)

    b_c = batch[safe(c_idx)]
    ch_c = chain[safe(c_idx)]
    r_c = resnum[safe(c_idx)]
    resi = np.zeros((B, NCH, NRES, n_alt), dtype=np.float32)
    ok2 = ((b_c >= 0) & (b_c < B) & (ch_c >= 0) & (ch_c < NCH)
           & (r_c >= 0) & (r_c < NRES))
    resi[b_c[ok2], ch_c[ok2], r_c[ok2], 0] = val[ok2]
    return resi


# ----------------------------------------------------------------- entry
def kernel(atom_description, coords, alternatives, weight, mean, std):
    ad = np.asarray(atom_description)
    co = np.asarray(coords, dtype=np.float32)
    al = np.asarray(alternatives)
    wt = np.asarray(weight, dtype=np.float32)
    mn = np.asarray(mean, dtype=np.float32)
    sd = np.asarray(std, dtype=np.float32)

    if _is_canonical(ad, co, mn, sd):
        return _run_fast(co, mn, sd, wt, al.shape[-1])
    return _fallback(ad, co, al, wt, mn, sd)
